# revision 1
# baseline (speedup 1.0000x reference)
"""Trainium2 Bass kernel for nn_BilinearScorer.

Computation (reference):
    pred [n=4096, h=512], args [n, h], U [h, R=64, h], bias1 [1, R*h], bias2 [1, R]
    first = pred @ U.reshape(h, R*h) + bias1           # [n, R*h]
    out   = einsum('nrk,nk->nr', first.reshape(n,R,h), args) + bias2   # [n, R]

Sharding: tensor-parallel over the role dim R. Each of the 8 cores owns
RL = 8 roles (its U / bias slice); pred and args are replicated. Each core
produces out[:, rc:rc+8]; the host concatenates. No collectives needed.

Per-core algorithm (all matmuls bf16 with fp32 PSUM accumulation):
  for each 128-token block b:
    C_psum[tok, r]  = sum_k args[tok,k] * bias1[r,k] + bias2[r]      (PE, N=8)
    for each local role r:
      F_psum[tok, k] = sum_j pred[tok,j] * U[j,r,k]                  (PE, 4 K-tiles)
      out[tok, r]    = reduce_k(F_psum * args) + C[tok, r]           (DVE fused
                       tensor_tensor_reduce, init scalar = C column)
"""

import numpy as np
import ml_dtypes

HID = 512
ROLES = 64
N_CORES = 8
RL = ROLES // N_CORES      # local roles per core
NTOK = 8 * 512             # b*t
P = 128                    # partitions
NBLK = NTOK // P           # 32 token blocks
JT = HID // P              # 4 contraction tiles (j)
KT = HID // P              # 4 contraction tiles (k)

_BF = ml_dtypes.bfloat16
_CACHE = {}


def _make_tile_context(nc):
    """TileContext whose kernel-tail drain splits its sem waits across
    multiple single-wait Drain instructions. The walrus build in this
    container rejects a Drain carrying >(about 2) sync waits
    (CoreV3GenImpl setupSyncWait: "Too many sync wait commands")."""
    import concourse.mybir as mybir
    from concourse.tile import TileContext
    from concourse.vector_clock import ScopedClock

    class SplitDrainTileContext(TileContext):
        # Max sync-waits this walrus accepts per instruction. Excess waits
        # are hoisted onto same-engine NoOps placed immediately before.
        _WAIT_LIMIT = 1

        def _commit_instruction(self, inst, lazy_reg_writes=True):
            limit = self._WAIT_LIMIT
            si = inst.sync_info
            if limit is not None and si is not None and len(si.on_wait) > limit:
                waits = list(si.on_wait)
                excess, keep = waits[:-limit], waits[-limit:]
                for w in excess:
                    noop = mybir.InstNoOp(
                        name=self.nc.get_next_instruction_name(),
                        sync_info=mybir.SyncInfo(on_wait=[w], on_update=[]),
                        bass_nofuse=True,
                        engine=inst.engine,
                    )
                    super()._commit_instruction(noop, lazy_reg_writes=False)
                inst.sync_info = mybir.SyncInfo(
                    on_wait=keep, on_update=list(si.on_update)
                )
            return super()._commit_instruction(inst, lazy_reg_writes)

        def _drain_and_barrier(self, tick_clock, wait_clock):
            nc = self.nc
            drain_inst = nc.sync.drain()
            wait_clock.add_sem_waits(
                drain_inst.ins, ScopedClock({None: tick_clock.global_clock})
            )
            si = drain_inst.ins.sync_info
            if si is not None and len(si.on_wait) > 1:
                waits = list(si.on_wait)
                drain_inst.ins.sync_info = mybir.SyncInfo(
                    on_wait=[waits[0]], on_update=list(si.on_update)
                )
                for w in waits[1:]:
                    d2 = nc.sync.drain()
                    d2.ins.sync_info = mybir.SyncInfo(on_wait=[w], on_update=[])
            nc.all_engine_barrier()
            assert self.sems is not None
            popped = nc._tile_sem_poison_stack.pop()
            assert popped is self._sem_poison
            nc.clear_and_free_semaphores(list(self.sems.allocated().values()))
            nc.all_engine_barrier()

    return SplitDrainTileContext(nc)


def _build():
    """Build the (single-program SPMD) Bass module."""
    import concourse.bass as bass
    import concourse.mybir as mybir

    f32 = mybir.dt.float32
    bf16 = mybir.dt.bfloat16
    nc = bass.Bass()

    # DRAM I/O. Layouts are host-prepped so every DMA is partition-friendly:
    #   predt[p, jt, n]     = pred[n, jt*128+p]          (bf16)
    #   u[p, jt*RL*HID + r*HID + k] = U[jt*128+p, rc+r, k] (bf16)
    #   args[n, k]                                        (f32, natural)
    #   argst[p, kt, n]     = args[n, kt*128+p]          (bf16)
    #   b1t[p, kt*RL + r]   = bias1_2d[rc+r, kt*128+p]   (bf16)
    #   b2[0, r]            = bias2[rc+r]                (bf16)
    predt = nc.declare_dram_parameter("predt", [P, JT, NTOK], bf16, isOutput=False)
    u = nc.declare_dram_parameter("u", [P, JT * RL * HID], bf16, isOutput=False)
    args = nc.declare_dram_parameter("args", [NTOK, HID], bf16, isOutput=False)
    argst = nc.declare_dram_parameter("argst", [P, KT, NTOK], bf16, isOutput=False)
    b1t = nc.declare_dram_parameter("b1t", [P, KT * 32], bf16, isOutput=False)
    b2 = nc.declare_dram_parameter("b2", [1, 32], bf16, isOutput=False)
    out = nc.declare_dram_parameter("out", [NTOK, RL], f32, isOutput=True)

    with _make_tile_context(nc) as tc:
        with (
            tc.tile_pool(name="const", bufs=1) as cpool,
            tc.tile_pool(name="pred", bufs=5) as ppool,
            tc.tile_pool(name="argsp", bufs=5) as apool,
            tc.tile_pool(name="argstp", bufs=4) as atpool,
            tc.tile_pool(name="outp", bufs=3) as opool,
            tc.tile_pool(name="misc", bufs=2) as mpool,
            tc.tile_pool(name="fps", bufs=8, space="PSUM") as fpsum,
        ):
            # Startup order matters: the tiny bias tensors and block-0 inputs
            # go at the head of the sync queue so the PE can start within a
            # few us; the 4 MiB U load is split across the sync and gpsimd
            # queues; steady-state block inputs stream on the scalar queue.
            b1t_sb = cpool.tile([P, KT * 32], bf16)
            nc.sync.dma_start(out=b1t_sb[:], in_=b1t[:])
            b2_sb = cpool.tile([1, 32], bf16)
            nc.sync.dma_start(out=b2_sb[:], in_=b2[:])
            ones_sb = cpool.tile([1, HID], bf16)
            nc.vector.memset(ones_sb[:], 1.0)
            # Transposed bias-correction table: c_all[p, b*32+r] = C[b*128+p, r]
            c_all = cpool.tile([P, NBLK * 32], bf16)

            # PE warmup: ~4us of dummy matmuls while the first DMAs land.
            # Fills the otherwise-idle preamble window with a full HAM SHORT
            # window of activity so the 2.4 GHz clock is unthrottled before
            # the first real matmul (saves ~8us of half-clock execution).
            warm_w = cpool.tile([P, P], bf16)
            nc.vector.memset(warm_w[:], 0.125)
            warm_rhs = cpool.tile([P, HID], bf16)
            nc.vector.memset(warm_rhs[:], 0.125)
            warm_ps = fpsum.tile([P, HID], f32, name="warm_ps", tag="fps_tile")
            for i in range(12):
                nc.tensor.matmul(
                    warm_ps[:],
                    warm_w[:],
                    warm_rhs[:],
                    start=(i == 0),
                    stop=(i == 11),
                )
            warm_out = mpool.tile([P, 1], f32, name="warm_out", tag="warm_out")
            nc.vector.tensor_reduce(
                out=warm_out[:],
                in_=warm_ps[:],
                axis=mybir.AxisListType.X,
                op=mybir.AluOpType.max,
            )

            argst_chunks = {}
            pa_sbs = {}

            def load_argst_chunk(c, eng):
                tok = slice(c * 4 * P, (c + 1) * 4 * P)
                t = atpool.tile(
                    [P, KT, 4 * P], bf16, name="argst_sb", tag="argst_sb", bufs=3
                )
                eng.dma_start(out=t[:], in_=argst[:, :, tok])
                argst_chunks[c] = t

            def load_pa(b, eng):
                tok = slice(b * P, (b + 1) * P)
                args_sb = apool.tile([P, HID], bf16, name="args_sb", tag="args_sb")
                eng.dma_start(out=args_sb[:], in_=args[tok, :])
                pred_sb = ppool.tile([P, JT, P], bf16, name="pred_sb", tag="pred_sb")
                eng.dma_start(out=pred_sb[:], in_=predt[:, :, tok])
                pa_sbs[b] = (pred_sb, args_sb)

            def ct_part(c):
                """C_T for one 4-block chunk: C_T[m, n] = sum_k b1tp[k,m] *
                argst[k, n] + b2[m], m = 32 padded roles (8 real), n = 512
                tokens. Transposed back into c_all via the DMA xbar."""
                argst_sb = argst_chunks.pop(c)
                ct_ps = fpsum.tile([32, HID], f32, name="ct_ps", tag="fps_tile")
                for kt in range(KT):
                    nc.tensor.matmul(
                        ct_ps[:],
                        b1t_sb[:, kt * 32:(kt + 1) * 32],
                        argst_sb[:, kt, :],
                        start=(kt == 0),
                        stop=False,
                    )
                nc.tensor.matmul(ct_ps[:], b2_sb[:], ones_sb[:], start=False, stop=True)
                ct_sb = mpool.tile([32, HID], bf16, name="ct_sb", tag="ct_sb", bufs=2)
                nc.scalar.copy(out=ct_sb[:], in_=ct_ps[:])
                for bb in range(4):
                    nc.scalar.dma_start_transpose(
                        out=c_all[:, (4 * c + bb) * 32:(4 * c + bb + 1) * 32],
                        in_=ct_sb[:, bb * P:(bb + 1) * P],
                    )

            # Startup: block-0 critical tensors first on the sync queue, U
            # split across the sync/gpsimd queues. argst (feeds the cheap C
            # matmuls that keep the PE warm during the U load) prefetches
            # CLOOK_C ahead on the scalar queue; the bulkier pred/args only
            # CLOOK_F ahead to limit HBM contention with the U load.
            CLOOK_F = 3
            load_pa(0, nc.sync)

            load_argst_chunk(0, nc.sync)
            seg = RL * HID
            u_sbs = []
            for jt in range(JT):
                u_t = cpool.tile(
                    [P, seg], bf16, name=f"u_sb{jt}", tag=f"u_sb{jt}"
                )
                # Each j-tile split across both queues so the earliest-needed
                # tiles arrive first at twice the single-queue rate.
                half = seg // 2
                nc.sync.dma_start(
                    out=u_t[:, :half], in_=u[:, jt * seg:jt * seg + half]
                )
                nc.gpsimd.dma_start(
                    out=u_t[:, half:], in_=u[:, jt * seg + half:(jt + 1) * seg]
                )
                u_sbs.append(u_t)

            load_argst_chunk(1, nc.sync)
            ct_part(0)
            # Second warmup burst: bridge the remaining U-load window so the
            # PE never idles long enough for HAM to re-throttle.
            warm_ps2 = fpsum.tile([P, HID], f32, name="warm_ps2", tag="fps_tile")
            for i in range(16):
                nc.tensor.matmul(
                    warm_ps2[:],
                    warm_w[:],
                    warm_rhs[:],
                    start=(i == 0),
                    stop=(i == 15),
                )
            warm_out2 = mpool.tile([P, 1], f32, name="warm_out", tag="warm_out")
            nc.vector.tensor_reduce(
                out=warm_out2[:],
                in_=warm_ps2[:],
                axis=mybir.AxisListType.X,
                op=mybir.AluOpType.max,
            )
            for b in range(1, CLOOK_F):
                load_pa(b, nc.scalar)

            for b in range(NBLK):
                tok = slice(b * P, (b + 1) * P)
                if b % 4 == 0 and b // 4 + 2 < NBLK // 4:
                    load_argst_chunk(b // 4 + 2, nc.sync)
                if b + CLOOK_F < NBLK:
                    load_pa(b + CLOOK_F, nc.scalar)
                pred_sb, args_sb = pa_sbs.pop(b)

                acc_sb = mpool.tile([P, RL], f32, name="acc_sb", tag="acc_sb")
                out_sb = opool.tile([P, RL], f32)
                dummy = mpool.tile([P, 1], f32)
                if b < 2:
                    # Early blocks run jt-outer: the first matmuls need only
                    # u_jt0, which arrives long before the rest of U.
                    pss = [
                        fpsum.tile([P, HID], f32, name="fps_tile", tag="fps_tile")
                        for _ in range(RL)
                    ]
                    for jt in range(JT):
                        for r in range(RL):
                            nc.tensor.matmul(
                                pss[r][:],
                                pred_sb[:, jt, :],
                                u_sbs[jt][:, r * HID:(r + 1) * HID],
                                start=(jt == 0),
                                stop=(jt == JT - 1),
                            )
                    for r in range(RL):
                        nc.vector.scalar_tensor_tensor(
                            out=dummy.broadcast_to([P, HID]),
                            in0=pss[r][:],
                            scalar=1.0,
                            in1=args_sb[:],
                            op0=mybir.AluOpType.mult,
                            op1=mybir.AluOpType.mult,
                            accum_out=acc_sb[:, r:r + 1],
                        )
                else:
                    # Role-outer: each role's 4 accumulating matmuls finish
                    # back-to-back so its DVE reduce starts immediately (the
                    # per-matmul LDWEIGHTS cost is identical either way).
                    for r in range(RL):
                        ps = fpsum.tile(
                            [P, HID], f32, name="fps_tile", tag="fps_tile"
                        )
                        for jt in range(JT):
                            nc.tensor.matmul(
                                ps[:],
                                pred_sb[:, jt, :],
                                u_sbs[jt][:, r * HID:(r + 1) * HID],
                                start=(jt == 0),
                                stop=(jt == JT - 1),
                            )
                        nc.vector.scalar_tensor_tensor(
                            out=dummy.broadcast_to([P, HID]),
                            in0=ps[:],
                            scalar=1.0,
                            in1=args_sb[:],
                            op0=mybir.AluOpType.mult,
                            op1=mybir.AluOpType.mult,
                            accum_out=acc_sb[:, r:r + 1],
                        )
                nc.vector.tensor_add(
                    out=out_sb[:],
                    in0=acc_sb[:],
                    in1=c_all[:, b * 32:b * 32 + RL],
                )
                nc.gpsimd.dma_start(out=out[tok, :], in_=out_sb[:])
                if b == 0:
                    ct_part(1)
                if b % 4 == 3 and b // 4 + 2 < NBLK // 4:
                    ct_part(b // 4 + 2)
    return nc


def _prep_in_maps(pred_input, args_input, U, bias1, bias2):
    pred = np.asarray(pred_input, np.float32).reshape(NTOK, HID)
    args = np.asarray(args_input, np.float32).reshape(NTOK, HID)
    U = np.asarray(U, np.float32)
    bias1_2d = np.asarray(bias1, np.float32).reshape(ROLES, HID)
    bias2_v = np.asarray(bias2, np.float32).reshape(ROLES)

    predt = np.ascontiguousarray(
        pred.T.reshape(JT, P, NTOK).transpose(1, 0, 2).astype(_BF)
    )
    argst = np.ascontiguousarray(
        args.T.reshape(KT, P, NTOK).transpose(1, 0, 2).astype(_BF)
    )
    args_c = np.ascontiguousarray(args.astype(_BF))

    in_maps = []
    for c in range(N_CORES):
        rc = c * RL
        u_prep = np.ascontiguousarray(
            U[:, rc:rc + RL, :]
            .reshape(JT, P, RL, HID)
            .transpose(1, 0, 2, 3)
            .reshape(P, JT * RL * HID)
            .astype(_BF)
        )
        # b1t[p, kt*32 + m] = bias1_2d[rc+m, kt*128+p] for m < RL, else 0
        b1t_small = (
            bias1_2d[rc:rc + RL].T.reshape(KT, P, RL).transpose(1, 0, 2)
        )  # [P, KT, RL]
        b1t_pad = np.zeros((P, KT, 32), np.float32)
        b1t_pad[:, :, :RL] = b1t_small
        b1t = np.ascontiguousarray(b1t_pad.reshape(P, KT * 32).astype(_BF))
        b2_pad = np.zeros((1, 32), np.float32)
        b2_pad[0, :RL] = bias2_v[rc:rc + RL]
        b2c = np.ascontiguousarray(b2_pad.astype(_BF))
        in_maps.append(
            {
                "predt": predt,
                "u": u_prep,
                "args": args_c,
                "argst": argst,
                "b1t": b1t,
                "b2": b2c,
            }
        )
    return in_maps


def run(inputs, trace=False):
    """Run on all 8 cores; returns (full_output, BassKernelResults)."""
    from concourse.bass_utils import run_bass_kernel_spmd

    if "nc" not in _CACHE:
        _CACHE["nc"] = _build()
    in_maps = _prep_in_maps(**inputs)
    res = run_bass_kernel_spmd(
        _CACHE["nc"], in_maps, core_ids=list(range(N_CORES)), trace=trace
    )
    full = np.concatenate(
        [np.asarray(r["out"], np.float32) for r in res.results], axis=1
    )
    return full, res


def kernel(pred_input, args_input, U, bias1, bias2):
    full, _ = run(
        {
            "pred_input": pred_input,
            "args_input": args_input,
            "U": U,
            "bias1": bias1,
            "bias2": bias2,
        }
    )
    return full



# revision 7
# speedup vs baseline: 1.1638x; 1.1638x over previous
"""Trainium2 Bass kernel for nn_BilinearScorer.

Computation (reference):
    pred [n=4096, h=512], args [n, h], U [h, R=64, h], bias1 [1, R*h], bias2 [1, R]
    first = pred @ U.reshape(h, R*h) + bias1           # [n, R*h]
    out   = einsum('nrk,nk->nr', first.reshape(n,R,h), args) + bias2   # [n, R]

Sharding: tensor-parallel over the role dim R. Each of the 8 cores owns
RL = 8 roles (its U / bias slice); pred and args are replicated. Each core
produces out[:, rc:rc+8]; the host concatenates. No collectives needed.

Per-core algorithm (all matmuls bf16 with fp32 PSUM accumulation):
  for each 128-token block b:
    for each local role r:
      F_psum[tok, k] = sum_j pred[tok,j] * U[j,r,k]          (PE, 4 K-tiles)
      roles 0-5: acc[tok,r] = reduce_k(F_psum * args)        (DVE fused STT)
      roles 6-7: Act copies F_psum -> SBUF bf16, Pool does the fused STT
    out[tok, 0:6] = acc[:,0:6] + C[tok,0:6]   (DVE)
    out[tok, 6:8] = acc[:,6:8] + C[tok,6:8]   (Pool)
  C[tok, r] = sum_k bias1[r,k]*args[tok,k] + bias2[r] computed by small
  PE matmuls (ct_part) per 4-block chunk, transposed into c_all via DMA.

Engine/queue plan: Sync queue streams pred/args blocks + output; Scalar
and GpSimd queues split the 4 MiB U load (jt-ordered so early blocks can
start); Vector queue loads argst chunks for ct_part. Startup fills the
U-load window with HAM warmup matmuls + ct_part(0,1) + a jt-outer block 0.
"""

import numpy as np
import ml_dtypes

HID = 512
ROLES = 64
N_CORES = 8
RL = ROLES // N_CORES      # local roles per core
NTOK = 8 * 512             # b*t
P = 128                    # partitions
NBLK = NTOK // P           # 32 token blocks
JT = HID // P              # 4 contraction tiles (j)
KT = HID // P              # 4 contraction tiles (k)

ROWTILE = False            # K=64 row-tiled matmul pairs (probe-gated)
N_OFF = 2                  # roles offloaded to Act-copy + Pool-STT path

_BF = ml_dtypes.bfloat16
_CACHE = {}


def _make_tile_context(nc):
    """TileContext whose kernel-tail drain splits its sem waits across
    multiple single-wait Drain instructions. The walrus build in this
    container rejects a Drain carrying >(about 2) sync waits
    (CoreV3GenImpl setupSyncWait: "Too many sync wait commands")."""
    import concourse.mybir as mybir
    from concourse.tile import TileContext
    from concourse.vector_clock import ScopedClock

    class SplitDrainTileContext(TileContext):
        # Max sync-waits this walrus accepts per instruction. Excess waits
        # are hoisted onto same-engine NoOps placed immediately before.
        _WAIT_LIMIT = 1

        def _commit_instruction(self, inst, lazy_reg_writes=True):
            limit = self._WAIT_LIMIT
            si = inst.sync_info
            if limit is not None and si is not None and len(si.on_wait) > limit:
                waits = list(si.on_wait)
                excess, keep = waits[:-limit], waits[-limit:]
                for w in excess:
                    noop = mybir.InstNoOp(
                        name=self.nc.get_next_instruction_name(),
                        sync_info=mybir.SyncInfo(on_wait=[w], on_update=[]),
                        bass_nofuse=True,
                        engine=inst.engine,
                    )
                    super()._commit_instruction(noop, lazy_reg_writes=False)
                inst.sync_info = mybir.SyncInfo(
                    on_wait=keep, on_update=list(si.on_update)
                )
            return super()._commit_instruction(inst, lazy_reg_writes)

        def _drain_and_barrier(self, tick_clock, wait_clock):
            nc = self.nc
            drain_inst = nc.sync.drain()
            wait_clock.add_sem_waits(
                drain_inst.ins, ScopedClock({None: tick_clock.global_clock})
            )
            si = drain_inst.ins.sync_info
            if si is not None and len(si.on_wait) > 1:
                waits = list(si.on_wait)
                drain_inst.ins.sync_info = mybir.SyncInfo(
                    on_wait=[waits[0]], on_update=list(si.on_update)
                )
                for w in waits[1:]:
                    d2 = nc.sync.drain()
                    d2.ins.sync_info = mybir.SyncInfo(on_wait=[w], on_update=[])
            nc.all_engine_barrier()
            assert self.sems is not None
            popped = nc._tile_sem_poison_stack.pop()
            assert popped is self._sem_poison
            nc.clear_and_free_semaphores(list(self.sems.allocated().values()))
            nc.all_engine_barrier()

    return SplitDrainTileContext(nc)


def _build():
    """Build the (single-program SPMD) Bass module."""
    import concourse.bass as bass
    import concourse.mybir as mybir

    f32 = mybir.dt.float32
    bf16 = mybir.dt.bfloat16
    nc = bass.Bass()

    # DRAM I/O. Layouts are host-prepped so every DMA is partition-friendly:
    #   predt[p, jt, n]     = pred[n, jt*128+p]          (bf16)
    #   u[p, jt*RL*HID + r*HID + k] = U[jt*128+p, rc+r, k] (bf16)
    #   args[n, k]                                        (bf16, natural)
    #   argst[p, kt, n]     = args[n, kt*128+p]          (bf16)
    #   b1t[p, kt*32 + r]   = bias1_2d[rc+r, kt*128+p]   (bf16)
    #   b2[0, r]            = bias2[rc+r]                (bf16)
    predt = nc.declare_dram_parameter("predt", [P, JT, NTOK], bf16, isOutput=False)
    u = nc.declare_dram_parameter("u", [P, JT * RL * HID], bf16, isOutput=False)
    args = nc.declare_dram_parameter("args", [NTOK, HID], bf16, isOutput=False)
    argst = nc.declare_dram_parameter("argst", [P, KT, NTOK], bf16, isOutput=False)
    b1t = nc.declare_dram_parameter("b1t", [P, KT * 32], bf16, isOutput=False)
    b2 = nc.declare_dram_parameter("b2", [1, 32], bf16, isOutput=False)
    out = nc.declare_dram_parameter("out", [NTOK, RL], f32, isOutput=True)

    with _make_tile_context(nc) as tc:
        with (
            tc.tile_pool(name="const", bufs=1) as cpool,
            tc.tile_pool(name="pred", bufs=6) as ppool,
            tc.tile_pool(name="argsp", bufs=6) as apool,
            tc.tile_pool(name="argstp", bufs=3) as atpool,
            tc.tile_pool(name="outp", bufs=3) as opool,
            tc.tile_pool(name="misc", bufs=2) as mpool,
            tc.tile_pool(name="fbp", bufs=3) as fbpool,
            tc.tile_pool(name="fps", bufs=8, space="PSUM") as fpsum,
        ):
            # ---- constants / small tensors (head of sync queue)
            b1t_sb = cpool.tile([P, KT * 32], bf16)
            nc.sync.dma_start(out=b1t_sb[:], in_=b1t[:])
            b2_sb = cpool.tile([1, 32], bf16)
            nc.sync.dma_start(out=b2_sb[:], in_=b2[:])
            ones_sb = cpool.tile([1, HID], bf16)
            nc.vector.memset(ones_sb[:], 1.0)
            # Transposed bias-correction table: c_all[p, b*32+r] = C[b*128+p, r]
            c_all = cpool.tile([P, NBLK * 32], bf16)

            warm_w = cpool.tile([P, P], bf16)
            nc.vector.memset(warm_w[:], 0.125)
            warm_rhs = cpool.tile([P, HID], bf16)
            nc.vector.memset(warm_rhs[:], 0.125)

            argst_chunks = {}
            pa_sbs = {}

            def load_argst_chunk(c, eng):
                tok = slice(c * 4 * P, (c + 1) * 4 * P)
                t = atpool.tile(
                    [P, KT, 4 * P], bf16, name="argst_sb", tag="argst_sb", bufs=3
                )
                eng.dma_start(out=t[:], in_=argst[:, :, tok])
                argst_chunks[c] = t

            def load_pa(b, eng):
                tok = slice(b * P, (b + 1) * P)
                args_sb = apool.tile([P, HID], bf16, name="args_sb", tag="args_sb")
                eng.dma_start(out=args_sb[:], in_=args[tok, :])
                pred_sb = ppool.tile([P, JT, P], bf16, name="pred_sb", tag="pred_sb")
                eng.dma_start(out=pred_sb[:], in_=predt[:, :, tok])
                pa_sbs[b] = (pred_sb, args_sb)

            # ---- startup DMA issue order (DMA queues: sync/SP, scalar/Act,
            # gpsimd/Pool). U split jt-ordered across scalar+gpsimd so u_jt0
            # lands first; everything else streams on sync.
            load_argst_chunk(0, nc.sync)
            load_pa(0, nc.sync)
            seg = RL * HID
            half = seg // 2
            u_sbs = []
            for jt in range(JT):
                u_t = cpool.tile([P, seg], bf16, name=f"u_sb{jt}", tag=f"u_sb{jt}")
                u_sbs.append(u_t)
            for jt in range(JT):
                nc.scalar.dma_start(
                    out=u_sbs[jt][:, :half], in_=u[:, jt * seg:jt * seg + half]
                )
                nc.gpsimd.dma_start(
                    out=u_sbs[jt][:, half:], in_=u[:, jt * seg + half:(jt + 1) * seg]
                )
            CLOOK_F = 4
            for b in range(1, 3):
                load_pa(b, nc.sync)
            load_argst_chunk(1, nc.sync)
            for b in range(3, CLOOK_F + 1):
                load_pa(b, nc.sync)
            load_argst_chunk(2, nc.sync)
            load_argst_chunk(3, nc.sync)

            def ct_part(c):
                """C_T for one 4-block chunk: C_T[m, n] = sum_k b1tp[k,m] *
                argst[k, n] + b2[m], m = 32 padded roles (8 real), n = 512
                tokens. Transposed back into c_all via the DMA xbar."""
                argst_sb = argst_chunks.pop(c)
                ct_ps = fpsum.tile([32, HID], f32, name="fps_tile", tag="fps_tile")
                for kt in range(KT):
                    nc.tensor.matmul(
                        ct_ps[:],
                        b1t_sb[:, kt * 32:(kt + 1) * 32],
                        argst_sb[:, kt, :],
                        start=(kt == 0),
                        stop=False,
                    )
                nc.tensor.matmul(ct_ps[:], b2_sb[:], ones_sb[:], start=False, stop=True)
                ct_sb = mpool.tile([32, HID], bf16, name="ct_sb", tag="ct_sb", bufs=2)
                nc.scalar.copy(out=ct_sb[:], in_=ct_ps[:])
                for bb in range(4):
                    nc.scalar.dma_start_transpose(
                        out=c_all[:, (4 * c + bb) * 32:(4 * c + bb + 1) * 32],
                        in_=ct_sb[:, bb * P:(bb + 1) * P],
                    )

            def role_matmuls(ps, pred_sb, r, start, stop):
                rs = slice(r * HID, (r + 1) * HID)
                if ROWTILE:
                    for jt in range(JT):
                        nc.tensor.matmul(
                            ps[:],
                            pred_sb[0:64, jt, :],
                            u_sbs[jt][0:64, rs],
                            start=start and (jt == 0),
                            stop=False,
                        )
                        nc.tensor.matmul(
                            ps[:],
                            pred_sb[64:128, jt, :],
                            u_sbs[jt][64:128, rs],
                            start=False,
                            stop=stop and (jt == JT - 1),
                        )
                else:
                    for jt in range(JT):
                        nc.tensor.matmul(
                            ps[:],
                            pred_sb[:, jt, :],
                            u_sbs[jt][:, rs],
                            start=start and (jt == 0),
                            stop=stop and (jt == JT - 1),
                        )

            def stage2(r, ps, args_sb, acc, dummy):
                """Fused multiply-reduce acc[:,r] = sum_k F*args. Roles
                >= RL-N_OFF route through an Act-engine PSUM->SBUF bf16 copy
                so the DVE STT reads two bf16 SBUF operands (2x pumped) and
                the PSUM bank frees earlier."""
                if r < RL - N_OFF:
                    in0 = ps
                else:
                    fb = fbpool.tile([P, HID], bf16, name="fb", tag="fb")
                    nc.scalar.copy(out=fb[:], in_=ps[:])
                    in0 = fb
                nc.vector.scalar_tensor_tensor(
                    out=dummy.broadcast_to([P, HID]),
                    in0=in0[:],
                    scalar=1.0,
                    in1=args_sb[:],
                    op0=mybir.AluOpType.mult,
                    op1=mybir.AluOpType.mult,
                    accum_out=acc[:, r:r + 1],
                )

            def block(b, jt_outer=False):
                tok = slice(b * P, (b + 1) * P)
                pred_sb, args_sb = pa_sbs.pop(b)
                acc = mpool.tile([P, RL], f32, name="acc_sb", tag="acc_sb")
                out_sb = opool.tile([P, RL], f32)
                dummy = mpool.tile([P, 1], f32)
                nsplit = RL - N_OFF
                if jt_outer:
                    pss = [
                        fpsum.tile([P, HID], f32, name="fps_tile", tag="fps_tile")
                        for _ in range(RL)
                    ]
                    for jt in range(JT):
                        for r in range(RL):
                            nc.tensor.matmul(
                                pss[r][:],
                                pred_sb[:, jt, :],
                                u_sbs[jt][:, r * HID:(r + 1) * HID],
                                start=(jt == 0),
                                stop=(jt == JT - 1),
                            )
                    for r in range(RL):
                        stage2(r, pss[r], args_sb, acc, dummy)
                else:
                    for r in range(RL):
                        ps = fpsum.tile(
                            [P, HID], f32, name="fps_tile", tag="fps_tile"
                        )
                        role_matmuls(ps, pred_sb, r, True, True)
                        stage2(r, ps, args_sb, acc, dummy)
                nc.vector.tensor_add(
                    out=out_sb[:],
                    in0=acc[:],
                    in1=c_all[:, b * 32:b * 32 + RL],
                )
                nc.sync.dma_start(out=out[tok, :], in_=out_sb[:])

            # ---- PE program: warmup -> ct0/ct1 -> block0 (jt-outer) -> steady
            warm_ps = fpsum.tile([P, 2 * P], f32, name="fps_tile", tag="fps_tile")
            for i in range(14):
                nc.tensor.matmul(
                    warm_ps[:],
                    warm_w[:],
                    warm_rhs[:, :2 * P],
                    start=(i == 0),
                    stop=(i == 13),
                )
            warm_out = mpool.tile([P, 1], f32, name="warm_out", tag="warm_out")
            nc.vector.tensor_reduce(
                out=warm_out[:],
                in_=warm_ps[:],
                axis=mybir.AxisListType.X,
                op=mybir.AluOpType.max,
            )
            ct_part(0)
            block(0, jt_outer=True)
            ct_part(1)

            for b in range(1, NBLK):
                if b % 4 == 0 and b >= 8 and b // 4 + 2 < NBLK // 4:
                    load_argst_chunk(b // 4 + 2, nc.sync)
                if b + CLOOK_F < NBLK:
                    load_pa(b + CLOOK_F, nc.sync)
                block(b)
                if b % 4 == 3 and b // 4 + 2 < NBLK // 4:
                    ct_part(b // 4 + 2)
    return nc


def _prep_in_maps(pred_input, args_input, U, bias1, bias2):
    pred = np.asarray(pred_input, np.float32).reshape(NTOK, HID)
    args = np.asarray(args_input, np.float32).reshape(NTOK, HID)
    U = np.asarray(U, np.float32)
    bias1_2d = np.asarray(bias1, np.float32).reshape(ROLES, HID)
    bias2_v = np.asarray(bias2, np.float32).reshape(ROLES)

    predt = np.ascontiguousarray(
        pred.T.reshape(JT, P, NTOK).transpose(1, 0, 2).astype(_BF)
    )
    argst = np.ascontiguousarray(
        args.T.reshape(KT, P, NTOK).transpose(1, 0, 2).astype(_BF)
    )
    args_c = np.ascontiguousarray(args.astype(_BF))

    in_maps = []
    for c in range(N_CORES):
        rc = c * RL
        u_prep = np.ascontiguousarray(
            U[:, rc:rc + RL, :]
            .reshape(JT, P, RL, HID)
            .transpose(1, 0, 2, 3)
            .reshape(P, JT * RL * HID)
            .astype(_BF)
        )
        # b1t[p, kt*32 + m] = bias1_2d[rc+m, kt*128+p] for m < RL, else 0
        b1t_small = (
            bias1_2d[rc:rc + RL].T.reshape(KT, P, RL).transpose(1, 0, 2)
        )  # [P, KT, RL]
        b1t_pad = np.zeros((P, KT, 32), np.float32)
        b1t_pad[:, :, :RL] = b1t_small
        b1t = np.ascontiguousarray(b1t_pad.reshape(P, KT * 32).astype(_BF))
        b2_pad = np.zeros((1, 32), np.float32)
        b2_pad[0, :RL] = bias2_v[rc:rc + RL]
        b2c = np.ascontiguousarray(b2_pad.astype(_BF))
        in_maps.append(
            {
                "predt": predt,
                "u": u_prep,
                "args": args_c,
                "argst": argst,
                "b1t": b1t,
                "b2": b2c,
            }
        )
    return in_maps


def run(inputs, trace=False):
    """Run on all 8 cores; returns (full_output, BassKernelResults)."""
    from concourse.bass_utils import run_bass_kernel_spmd

    if "nc" not in _CACHE:
        _CACHE["nc"] = _build()
    in_maps = _prep_in_maps(**inputs)
    res = run_bass_kernel_spmd(
        _CACHE["nc"], in_maps, core_ids=list(range(N_CORES)), trace=trace
    )
    full = np.concatenate(
        [np.asarray(r["out"], np.float32) for r in res.results], axis=1
    )
    return full, res


def kernel(pred_input, args_input, U, bias1, bias2):
    full, _ = run(
        {
            "pred_input": pred_input,
            "args_input": args_input,
            "U": U,
            "bias1": bias1,
            "bias2": bias2,
        }
    )
    return full
